# revision 1
# baseline (speedup 1.0000x reference)
"""
MinibatchDiscrimination kernel for 8x TRN2 NeuronCores (Bass/Tile).

Math:  x = inputs @ T  -> [B, K, D] with B=512, K=100, D=5
       out[i,k] = sum_j exp(-sum_d |x[i,k,d]-x[j,k,d]|)

Strategy (per core c of 8):
  - Host passes, per core: inputsT_c = (roll(inputs, -64c, axis=0)).T as fp16
    [F, B], T as fp16 [F, KD], plus small constant matrices. Rolling the
    batch axis makes the program SPMD-identical: every core computes output
    rows for "columns 0..63" of its own xT.
  - Device: xT[kd, i] = sum_f T[f, kd] * inputsT[f, i]   (PE, 4 chunks of 125)
    S[k, i] = sum_d x[i,k,d]  (PE ones-block matmul over xT, stored fp16)
  - Per output row j in 0..63, using |t| = 2*relu(t) - t:
      rl_c[p,i]   = relu(xT_c[p,i] - xT_c[p,j])   (DVE tensor_scalar
                                                   (subtract, max 0.0); the
                                                   per-partition scalar is an
                                                   f32 upcast of the fp16 xT
                                                   column so the diagonal is
                                                   exactly 0)
      dist[:, i]  = -S[k,i] + 2*sum_d rl           (PE: negI matmul into psum +
                                                    2.0-block col-tiled matmuls)
      raw[:, j]   = sum_i exp(-dist[:,i])          (ACT fused exp + accum_out,
                                                    no bias)
      out         = raw * exp(-S16[:, 0:64])       (one DVE multiply at the
                                                    end: the per-partition
                                                    exp(-S_kj) factor is
                                                    constant over i, so it
                                                    factors out of the sum)
    since sum_d |diff| = 2*sum_d relu(diff) - S_ki + S_kj, and the S terms
    cancel exactly on the diagonal.
  - dist row p=32c+m holds k=25c+m (m<25); host transposes/reassembles.

  Hardware notes baked into the structure (measured on TRN2):
  - Compute instructions carry at most ONE semaphore wait after bacc's
    split pass; persistent manually-rotated tiles (dist/dump/ab) keep
    cross-iteration WAR deps same-engine so waits stay within budget.
  - The pipeline is ACT/DVE-bound: exp+accum ~= 0.85us per row; DVE
    (4 tensor_scalars ~1us) and PE (~0.9us, partially col-tile-overlapped)
    overlap with it. Measured steady state ~64-72us for the 64-row loop
    (axon-link wall-clock noise is +-10us; best measured 63.8us).
    GPSIMD offload of a relu chunk was tried and is ~7x slower than DVE
    on the Q7 path -- do not route tensor_scalar to gpsimd here.
    Also measured as no-wins (within +-10us link noise): deeper ab/dist
    buffering (12/5), exp main-out to SBUF fp16 instead of PSUM f32,
    and a rank-1 PE matmul replacing the exp bias (that one regressed
    ~40% -- K=1 matmuls serialize on the PE critical path).
  - Residual overhead, quantified from the final IR: each relu
    tensor_scalar carries a redundant same-engine WAW wait (ab-buffer
    rotation) in addition to its real PE WAR; bacc's 1-wait limit splits
    it into an EventSemaphore on the DVE queue -- 242 of them, ~10us of
    issue time on the bottleneck engine. Eliding same-engine WAW sems in
    Tile/bacc would recover most of the gap to the ~60us arithmetic floor.
  - The input stage (DMA 2MB fp16, xT matmuls, S row-sums) overlaps the
    loop start; the ACT exp table is pre-warmed during the DMAs.

By symmetry of the distance matrix, summing exp(-dist) over the free axis i
for a fixed row j gives exactly out[j, k] (self term included).
"""

import sys
import numpy as np

for _p in ("/opt/trn_rl_repo",):
    if _p not in sys.path:
        sys.path.insert(0, _p)

B = 512
F = 1024
K = 100
D = 5
KD = K * D  # 500
NCORES = 8
JPC = B // NCORES  # 64 output rows per core
NCHUNK = 4  # kd chunks of 125
CHUNK = KD // NCHUNK  # 125
KPC = K // NCHUNK  # 25 k's per chunk

_NC_CACHE = {}


def build_nc(bench_reps=1, ablate=()):
    import contextlib

    import concourse.bass as bass
    import concourse.bacc as bacc
    import concourse.mybir as mybir
    from concourse.tile import TileContext

    nc = bacc.Bacc(None, target_bir_lowering=False, debug=True)

    inT = nc.declare_dram_parameter("inT", [F, B], mybir.dt.float16, isOutput=False)
    Tm = nc.declare_dram_parameter("Tm", [F, KD], mybir.dt.float16, isOutput=False)
    # [:, 0:32] 2.0-valued d-sum block, [:, 32:64] 1.0-valued d-sum block
    onesd = nc.declare_dram_parameter(
        "onesd", [CHUNK, 64], mybir.dt.float16, isOutput=False
    )
    negI = nc.declare_dram_parameter("negI", [128, 128], mybir.dt.float16, isOutput=False)
    out = nc.declare_dram_parameter("out", [128, JPC], mybir.dt.float32, isOutput=True)

    with TileContext(nc) as tc:
        with tc.tile_pool(name="persist", bufs=1) as pp:
            T_sb = pp.tile([128, 8 * KD], mybir.dt.float16, name="T_sb")
            inT_sb = pp.tile([128, 8 * B], mybir.dt.float16, name="inT_sb")
            ones_sb = pp.tile([CHUNK, 64], mybir.dt.float16, name="ones_sb")
            negI_sb = pp.tile([128, 128], mybir.dt.float16, name="negI_sb")
            out_sb = pp.tile([128, JPC], mybir.dt.float32, name="out_sb")
            xT_sb = pp.tile([128, NCHUNK * B], mybir.dt.float16, name="xT_sb")
            # f32 upcasts of xT columns 0..JPC (tensor_scalar per-partition
            # scalars must be f32). Upcast from the fp16 xT so the diagonal
            # max(x,x)-x stays exactly zero.
            xTj_sb = pp.tile([128, NCHUNK * JPC], mybir.dt.float32, name="xTj_sb")
            S16_sb = pp.tile([128, B], mybir.dt.float16, name="S16_sb")
            expS_sb = pp.tile([128, JPC], mybir.dt.float32, name="expS_sb")
            raw_sb = pp.tile([128, JPC], mybir.dt.float32, name="raw_sb")

            # warm the ACT exp table while DMAs run (table load ~2.7us)
            warm_sb = pp.tile([1, 1], mybir.dt.float32, name="warm_sb")
            nc.vector.memset(warm_sb[:, :], 0.0)
            nc.scalar.activation(
                warm_sb[:, :], warm_sb[:, :], mybir.ActivationFunctionType.Exp
            )

            # --- load inputs ---
            for t in range(8):
                nc.sync.dma_start(
                    out=T_sb[:, t * KD : (t + 1) * KD],
                    in_=Tm[t * 128 : (t + 1) * 128, :],
                )
                nc.sync.dma_start(
                    out=inT_sb[:, t * B : (t + 1) * B],
                    in_=inT[t * 128 : (t + 1) * 128, :],
                )
            nc.sync.dma_start(out=ones_sb[:, :], in_=onesd[:, :])
            nc.sync.dma_start(out=negI_sb[:, :], in_=negI[:, :])

            with tc.tile_pool(name="xtps", bufs=2, space="PSUM") as xtps:
                # --- xT chunks: xT[kd, i] via PE over f tiles ---
                for c in range(NCHUNK):
                    xt_ps = xtps.tile([CHUNK, B], mybir.dt.float32, name="xt_ps")
                    for t in range(8):
                        nc.tensor.matmul(
                            xt_ps[:, :],
                            T_sb[:, t * KD + c * CHUNK : t * KD + (c + 1) * CHUNK],
                            inT_sb[:, t * B : (t + 1) * B],
                            start=(t == 0),
                            stop=(t == 7),
                        )
                    nc.scalar.copy(xT_sb[0:CHUNK, c * B : (c + 1) * B], xt_ps[:, :])
                    nc.vector.tensor_copy(
                        xTj_sb[0:CHUNK, c * JPC : (c + 1) * JPC],
                        xT_sb[0:CHUNK, c * B : c * B + JPC],
                    )

                # --- S[k, i] = sum_d x[i,k,d], arranged at partitions 32c+m ---
                S_ps = xtps.tile([128, B], mybir.dt.float32, name="S_ps", bufs=1)
                for c in range(NCHUNK):
                    nc.tensor.matmul(
                        S_ps[32 * c : 32 * c + 32, :],
                        ones_sb[:, 32:64],
                        xT_sb[0:CHUNK, c * B : (c + 1) * B],
                        start=True,
                        stop=True,
                        tile_position=(0, 32 * c),
                    )
                nc.vector.tensor_copy(S16_sb[:, :], S_ps[:, :])
                # exp(-S16[:, j]) factors: the per-partition exp bias is
                # constant over i, so it moves out of the accumulated sum
                # and becomes one elementwise multiply at the end.
                nc.scalar.activation(
                    expS_sb[:, :],
                    S16_sb[:, 0:JPC],
                    mybir.ActivationFunctionType.Exp,
                    bias=0.0,
                    scale=-1.0,
                )

            mainps_es = contextlib.ExitStack()
            mainps = mainps_es.enter_context(
                tc.tile_pool(name="mainps", bufs=1, space="PSUM")
            )

            # Persistent, manually double-buffered psum tiles. Persistent
            # (vs pool-rotated) so cross-iteration WAR deps are plain data
            # deps on fixed tiles: same-engine deps then cost no semaphore,
            # which matters because instructions carry at most ONE wait.
            dist_bufs = [
                mainps.tile([128, B], mybir.dt.float32, name=f"dist{i}") for i in range(4)
            ]
            dump_bufs = [
                mainps.tile([128, B], mybir.dt.float32, name=f"dump{i}") for i in range(2)
            ]
            # Persistent relu tiles, manually rotated (same reason).
            NAB = 8
            ab_bufs = [
                pp.tile([CHUNK, B], mybir.dt.float16, name=f"ab{i}") for i in range(NAB)
            ]

            # --- main loop over output rows ---
            # bench_reps>1 wraps the j-loop in a dynamic For_i so one NEFF
            # execution repeats the steady-state body (timing harness only).
            if ablate:
                # one unablated pass so every tile has a writer
                main_loop(nc, mybir, xT_sb, xTj_sb, S16_sb, raw_sb, negI_sb,
                          ones_sb, out_sb, dist_bufs, dump_bufs, ab_bufs)
            rep_ctx = (
                tc.For_i(0, bench_reps, 1) if bench_reps > 1 else contextlib.nullcontext()
            )
            with rep_ctx:
                main_loop(nc, mybir, xT_sb, xTj_sb, S16_sb, raw_sb, negI_sb,
                          ones_sb, out_sb, dist_bufs, dump_bufs, ab_bufs, ablate)

            # out = raw_sums * exp(-S16[:, j]) (the factored-out bias)
            nc.vector.tensor_tensor(
                out_sb[:, :], raw_sb[:, :], expS_sb[:, :], mybir.AluOpType.mult
            )
            mainps_es.close()
            nc.sync.dma_start(out=out[:, :], in_=out_sb[:, :])

    nc.finalize()
    return nc


def main_loop(nc, mybir, xT_sb, xTj_sb, S16_sb, raw_sb, negI_sb, ones_sb,
              out_sb, dist_bufs, dump_bufs, ab_bufs, ablate=()):
    NAB = len(ab_bufs)
    if True:
            for j in range(JPC):
                dist = dist_bufs[j % 4]
                # dist = -S[k, i]; also the first touch of dist this
                # iteration, absorbing the WAR-vs-ACT(exp of j-2) wait.
                if "mms" not in ablate and "mm" not in ablate:
                    nc.tensor.matmul(
                        dist[:, :],
                        negI_sb[:, :],
                        S16_sb[:, :],
                        start=True,
                        stop=False,
                        skip_group_check=True,
                    )
                for c in range(NCHUNK):
                    ab = ab_bufs[(j * NCHUNK + c) % NAB]
                    # relu(x_i - x_j) = max(x_i, x_j) - x_j
                    if "ts" not in ablate:
                        # relu(x_i - x_j) = (x_i - x_j) max 0; const scalar2
                        # keeps the second DVE read port free for 2x_2p/4x.
                        s1 = (
                            0.5
                            if "tsconst" in ablate
                            else xTj_sb[0:CHUNK, c * JPC + j : c * JPC + j + 1]
                        )
                        nc.vector.tensor_scalar(
                            ab[:, :],
                            xT_sb[0:CHUNK, c * B : (c + 1) * B],
                            s1,
                            0.0,
                            mybir.AluOpType.subtract,
                            mybir.AluOpType.max,
                        )
                    # dist[32c+m, :] += 2 * sum_d ab[5m+d, :]
                    if "mm" not in ablate:
                        nc.tensor.matmul(
                            dist[32 * c : 32 * c + 32, :],
                            ones_sb[:, 0:32],
                            ab[:, :],
                            start=("mms" in ablate and c == 0),
                            stop=(c == NCHUNK - 1),
                            tile_position=(0, 32 * c),
                            skip_group_check=True,
                        )
                dump = dump_bufs[j % 2]
                # out_sb[:, j] = sum_i exp(-dist[:, i] - S16[:, j])
                if "exp" not in ablate:
                    nc.scalar.activation(
                        dump[:, :],
                        dist[:, :],
                        mybir.ActivationFunctionType.Exp,
                        bias=0.0,
                        scale=-1.0,
                        accum_out=(None if "noaccum" in ablate else raw_sb[:, j : j + 1]),
                    )


def _aux_consts():
    ob = np.zeros([CHUNK, 64], dtype=np.float16)
    for m in range(KPC):
        ob[5 * m : 5 * m + 5, m] = 2.0
        ob[5 * m : 5 * m + 5, 32 + m] = 1.0
    negI = (-np.eye(128)).astype(np.float16)
    return ob, negI


def make_in_maps(inputs, T):
    f16 = np.float16
    Tm = np.asarray(T, dtype=np.float32).astype(f16)
    ob, negI = _aux_consts()
    in_maps = []
    for c in range(NCORES):
        rolled = np.roll(np.asarray(inputs, dtype=np.float32), -JPC * c, axis=0)
        inTc = np.ascontiguousarray(rolled.T).astype(f16)
        in_maps.append(
            {
                "inT": inTc,
                "Tm": Tm,
                "onesd": ob,
                "negI": negI,
            }
        )
    return in_maps


def assemble_output(results):
    out = np.zeros([B, K], dtype=np.float32)
    for c in range(NCORES):
        arr = np.asarray(results[c]["out"], dtype=np.float32)  # [128, JPC]
        for cc in range(NCHUNK):
            out[JPC * c : JPC * (c + 1), KPC * cc : KPC * (cc + 1)] = arr[
                32 * cc : 32 * cc + KPC, :
            ].T
    return out


def kernel(inputs, T):
    from concourse.bass_utils import run_bass_kernel_spmd

    if "nc" not in _NC_CACHE:
        _NC_CACHE["nc"] = build_nc()
    nc = _NC_CACHE["nc"]
    in_maps = make_in_maps(inputs, T)
    res = run_bass_kernel_spmd(nc, in_maps, list(range(NCORES)))
    return assemble_output(res.results)


if __name__ == "__main__":
    sys.path.insert(0, "/root/problem")
    from reference import setup_inputs, reference

    inputs = setup_inputs()
    expected = np.asarray(reference(**inputs))
    actual = kernel(**{k: np.asarray(v) for k, v in inputs.items()})
    err = np.abs(actual - expected)
    rel = np.linalg.norm(actual - expected) / np.linalg.norm(expected)
    print(f"max abs err: {err.max():.3e}")
    print(f"Relative error: {rel:.3e}")



# revision 10
# speedup vs baseline: 1.5669x; 1.5669x over previous
"""
MinibatchDiscrimination kernel for 8x TRN2 NeuronCores (Bass/Tile).

Math:  x = inputs @ T  -> [B, K, D] with B=512, K=100, D=5
       out[i,k] = sum_j exp(-sum_d |x[i,k,d]-x[j,k,d]|)

Strategy -- symmetric half-coverage (e_ij = e_ji):
  The batch is split into 8 blocks of 64 rows. Core c owns output rows
  j in block c, but computes e_ij only for i in blocks c..c+4 (cyclic),
  NI=320 columns instead of 512. Every unordered block pair {a, b} is
  covered exactly once: pairs at cyclic distance d in 1..3 by the lower
  core, d=4 pairs by cores 0..3 only -- cores 4..7 receive +1024-offset
  dummy data for their 5th block so those e values underflow to exactly
  0.0 (their row sums gain nothing, their d=4 column partials are 0).
  Each core emits BOTH row sums (ACT accum_out over the free axis) and
  per-column partial sums over its 64 rows (elementwise accumulation of
  the exp tiles); the host combines: out[rows of block c] = row sums of
  core c + column partials of cores c-1, c-2, c-3 (and c-4 for c>=4)
  at the matching columns. Within-block (d=0) pairs are fully covered by
  the row sums; the d=0 column-partial slice is simply not used.

Per-core device program (SPMD via host-side roll of the batch axis):
  - xT[kd, i] = sum_f T[f, kd] * inputsT[f, i]  (PE, 4 chunks of 125, NI cols)
  - S[k, i] = sum_d x[i,k,d]  (PE 1.0-ones-block matmuls, fp16 S16)
  - per j in 0..63, using |t| = 2*relu(t) - t:
      dist = -S_i (negI matmul) + 2*sum_d relu(x_i - x_j)
             (DVE tensor_scalar (subtract, max 0.0) per chunk + PE
              2.0-ones-block col-tiled matmuls; the per-partition scalar
              is an f32 upcast of the fp16 xT column so the diagonal is
              exactly 0)
      e[:, i] = exp(-dist - S_j)  via ACT exp with scale=-1 and
              per-partition bias AP = -S16[:, j] (f32); since
              sum_d |diff| = 2*sum_d relu - S_i + S_j the S terms cancel
              exactly on the diagonal (self term = exp(0) = 1).
      row sums: ACT accum_out -> out_sb[:, j]
      col partials: out_sb[:, 64:384]  += e  (Pool/GPSIMD, even j)
                    out_sb[:, 384:704] += e  (DVE, odd j)
              two independent same-engine accumulation chains; the host
              adds the two halves.
  - dist row p=32c+m holds k=25c+m (m<25); host transposes/reassembles.

  Cost-model balance per j (NI=320): PE = negI + 4 d-sum matmuls ~670ns,
  ACT = exp+accum ~640ns, DVE = 4 tensor_scalar (4x_2p) + odd-j col
  accum ~690ns, Pool ~365ns avg. The xT/S input stage is ~4.8us of PE.

  Hardware notes baked into the structure (measured on TRN2):
  - Compute instructions carry at most ONE semaphore wait after bacc's
    split pass; persistent manually-rotated tiles (dist/dump/ab) keep
    cross-iteration WAR deps same-engine so waits stay within budget.
"""

import sys
import numpy as np

for _p in ("/opt/trn_rl_repo",):
    if _p not in sys.path:
        sys.path.insert(0, _p)

B = 512
F = 1024
K = 100
D = 5
KD = K * D  # 500
NCORES = 8
JPC = B // NCORES  # 64 output rows per core
NBLK = 5  # i-blocks covered per core
NI = NBLK * JPC  # 320 i columns per core
NCHUNK = 4  # kd chunks of 125
CHUNK = KD // NCHUNK  # 125
KPC = K // NCHUNK  # 25 k's per chunk
DUMMY_OFF = 1024.0  # offset applied to block-4 inputs on cores 4..7

_NC_CACHE = {}


def build_nc(bench_reps=1, ablate=()):
    import contextlib

    import concourse.bass as bass
    import concourse.bacc as bacc
    import concourse.mybir as mybir
    from concourse.tile import TileContext

    nc = bacc.Bacc(None, target_bir_lowering=False, debug=True)

    inT = nc.declare_dram_parameter("inT", [F, NI], mybir.dt.float16, isOutput=False)
    Tm = nc.declare_dram_parameter("Tm", [F, KD], mybir.dt.float16, isOutput=False)
    # [:, 0:32] 2.0-valued d-sum block, [:, 32:64] 1.0-valued d-sum block
    onesd = nc.declare_dram_parameter(
        "onesd", [CHUNK, 64], mybir.dt.float16, isOutput=False
    )
    negI = nc.declare_dram_parameter("negI", [128, 128], mybir.dt.float16, isOutput=False)
    # [:, 0:64] row sums; [:, 64:384] even-j col partials; [:, 384:704] odd-j
    out = nc.declare_dram_parameter(
        "out", [128, JPC + 2 * NI], mybir.dt.float32, isOutput=True
    )

    with TileContext(nc) as tc:
        with tc.tile_pool(name="persist", bufs=1) as pp:
            T_sb = pp.tile([128, 8 * KD], mybir.dt.float16, name="T_sb")
            inT_sb = pp.tile([128, 8 * NI], mybir.dt.float16, name="inT_sb")
            ones_sb = pp.tile([CHUNK, 64], mybir.dt.float16, name="ones_sb")
            negI_sb = pp.tile([128, 128], mybir.dt.float16, name="negI_sb")
            out_sb = pp.tile([128, JPC + 2 * NI], mybir.dt.float32, name="out_sb")
            xT_sb = pp.tile([128, NCHUNK * NI], mybir.dt.float16, name="xT_sb")
            # f32 upcasts of xT columns 0..JPC (tensor_scalar per-partition
            # scalars must be f32). Upcast from the fp16 xT so the diagonal
            # subtract is exactly zero.
            xTj_sb = pp.tile([128, NCHUNK * JPC], mybir.dt.float32, name="xTj_sb")
            S16_sb = pp.tile([128, NI], mybir.dt.float16, name="S16_sb")
            # -S16[:, 0:64] upcast to f32: per-partition exp bias APs
            negSj_sb = pp.tile([128, JPC], mybir.dt.float32, name="negSj_sb")

            # warm the ACT exp table while DMAs run (table load ~1.3us)
            warm_sb = pp.tile([1, 1], mybir.dt.float32, name="warm_sb")
            nc.vector.memset(warm_sb[:, :], 0.0)
            nc.scalar.activation(
                warm_sb[:, :], warm_sb[:, :], mybir.ActivationFunctionType.Exp
            )
            # zero the two column-partial accumulator regions
            nc.gpsimd.memset(out_sb[:, JPC : JPC + 2 * NI], 0.0)

            # --- load inputs ---
            # T/inT in 2-f-tile groups, interleaved, so the PE can start
            # consuming f-tile pairs while later pairs still transfer (each
            # HWDGE issue is ~625ns on a device-global queue, so fewer DMAs
            # beat per-tile ones). The tiny consts go via the Pool SWDGE
            # path, which doesn't contend with HWDGE.
            for g in range(4):
                t0 = 2 * g
                nc.sync.dma_start(
                    out=T_sb[:, t0 * KD : (t0 + 2) * KD].rearrange(
                        "p (g c) -> p g c", g=2
                    ),
                    in_=Tm[t0 * 128 : (t0 + 2) * 128, :].rearrange(
                        "(g p) c -> p g c", g=2
                    ),
                )
                nc.sync.dma_start(
                    out=inT_sb[:, t0 * NI : (t0 + 2) * NI].rearrange(
                        "p (g c) -> p g c", g=2
                    ),
                    in_=inT[t0 * 128 : (t0 + 2) * 128, :].rearrange(
                        "(g p) c -> p g c", g=2
                    ),
                )
            nc.gpsimd.dma_start(out=ones_sb[:, :], in_=onesd[:, :])
            nc.gpsimd.dma_start(out=negI_sb[:, :], in_=negI[:, :])

            with tc.tile_pool(name="xtps", bufs=1, space="PSUM") as xtps:
                # --- xT chunks: xT[kd, i] via PE, f-tile-group outer so the
                # PE consumes DMA groups as they land ---
                xt_ps_l = [
                    xtps.tile([CHUNK, NI], mybir.dt.float32, name=f"xt_ps{c}")
                    for c in range(NCHUNK)
                ]
                for g in range(4):
                    for c in range(NCHUNK):
                        for dt_ in range(2):
                            t = 2 * g + dt_
                            nc.tensor.matmul(
                                xt_ps_l[c][:, :],
                                T_sb[:, t * KD + c * CHUNK : t * KD + (c + 1) * CHUNK],
                                inT_sb[:, t * NI : (t + 1) * NI],
                                start=(t == 0),
                                stop=(t == 7),
                            )
                        if g == 3:
                            # chunk c complete: copy out while later chunks
                            # finish (alternate ACT/DVE to halve the serial
                            # copy tail)
                            if c % 2 == 0:
                                nc.scalar.copy(
                                    xT_sb[0:CHUNK, c * NI : (c + 1) * NI],
                                    xt_ps_l[c][:, :],
                                )
                            else:
                                nc.vector.tensor_copy(
                                    xT_sb[0:CHUNK, c * NI : (c + 1) * NI],
                                    xt_ps_l[c][:, :],
                                )
                            nc.vector.tensor_copy(
                                xTj_sb[0:CHUNK, c * JPC : (c + 1) * JPC],
                                xT_sb[0:CHUNK, c * NI : c * NI + JPC],
                            )

                # --- S[k, i] = sum_d x[i,k,d], arranged at partitions 32c+m ---
                S_ps = xtps.tile([128, NI], mybir.dt.float32, name="S_ps", bufs=1)
                for c in range(NCHUNK):
                    nc.tensor.matmul(
                        S_ps[32 * c : 32 * c + 32, :],
                        ones_sb[:, 32:64],
                        xT_sb[0:CHUNK, c * NI : (c + 1) * NI],
                        start=True,
                        stop=True,
                        tile_position=(0, 32 * c),
                    )
                nc.vector.tensor_copy(S16_sb[:, :], S_ps[:, :])
                # f32 upcast of -S16[:, j]: exact negation of the fp16 values
                # so exp(-dist - S_j) is exactly exp(0)=1 on the diagonal.
                nc.vector.tensor_scalar(
                    negSj_sb[:, :],
                    S16_sb[:, 0:JPC],
                    -1.0,
                    None,
                    mybir.AluOpType.mult,
                )

            mainps_es = contextlib.ExitStack()
            mainps = mainps_es.enter_context(
                tc.tile_pool(name="mainps", bufs=1, space="PSUM")
            )

            # Persistent, manually double-buffered psum tiles. Persistent
            # (vs pool-rotated) so cross-iteration WAR deps are plain data
            # deps on fixed tiles: same-engine deps then cost no semaphore,
            # which matters because instructions carry at most ONE wait.
            dist_bufs = [
                mainps.tile([128, NI], mybir.dt.float32, name=f"dist{i}")
                for i in range(4)
            ]
            dump_bufs = [
                pp.tile([128, NI], mybir.dt.float32, name=f"dump{i}") for i in range(2)
            ]
            # Persistent relu tiles, manually rotated (same reason).
            NAB = 8
            ab_bufs = [
                pp.tile([CHUNK, NI], mybir.dt.float16, name=f"ab{i}") for i in range(NAB)
            ]

            rep_ctx = (
                tc.For_i(0, bench_reps, 1) if bench_reps > 1 else contextlib.nullcontext()
            )
            with rep_ctx:
                main_loop(nc, mybir, xT_sb, xTj_sb, S16_sb, negSj_sb, negI_sb,
                          ones_sb, out_sb, dist_bufs, dump_bufs, ab_bufs, ablate)

            mainps_es.close()
            nc.gpsimd.dma_start(out=out[:, :], in_=out_sb[:, :])

    nc.finalize()
    return nc


def main_loop(nc, mybir, xT_sb, xTj_sb, S16_sb, negSj_sb, negI_sb, ones_sb,
              out_sb, dist_bufs, dump_bufs, ab_bufs, ablate=()):
    NAB = len(ab_bufs)
    for j in range(JPC):
        dist = dist_bufs[j % 4]
        # dist = -S[k, i]; also the first touch of dist this iteration,
        # absorbing the WAR-vs-ACT(exp of j-4) wait.
        if "mm" not in ablate:
            nc.tensor.matmul(
                dist[:, :],
                negI_sb[:, :],
                S16_sb[:, :],
                start=True,
                stop=False,
                skip_group_check=True,
            )
        for c in range(NCHUNK):
            ab = ab_bufs[(j * NCHUNK + c) % NAB]
            if "ts" not in ablate:
                # relu(x_i - x_j) = (x_i - x_j) max 0; const scalar2
                # keeps the second DVE read port free for 4x_2p mode.
                nc.vector.tensor_scalar(
                    ab[:, :],
                    xT_sb[0:CHUNK, c * NI : (c + 1) * NI],
                    xTj_sb[0:CHUNK, c * JPC + j : c * JPC + j + 1],
                    0.0,
                    mybir.AluOpType.subtract,
                    mybir.AluOpType.max,
                )
            # dist[32c+m, :] += 2 * sum_d ab[5m+d, :]
            if "mm" not in ablate:
                nc.tensor.matmul(
                    dist[32 * c : 32 * c + 32, :],
                    ones_sb[:, 0:32],
                    ab[:, :],
                    start=False,
                    stop=(c == NCHUNK - 1),
                    tile_position=(0, 32 * c),
                    skip_group_check=True,
                )
        dump = dump_bufs[j % 2]
        # e = exp(-dist - S_j); row sums via accum_out
        if "exp" not in ablate:
            nc.scalar.activation(
                dump[:, :],
                dist[:, :],
                mybir.ActivationFunctionType.Exp,
                bias=negSj_sb[:, j : j + 1],
                scale=-1.0,
                accum_out=(None if "noaccum" in ablate else out_sb[:, j : j + 1]),
            )
        # column partials: two independent Pool accumulation chains
        # (Pool tensor_tensor is ~267ns/call in the cost model and the
        # engine is otherwise idle; DVE f32 tensor_tensor gets no 2x mode)
        if "colacc" not in ablate:
            half = JPC if j % 2 == 0 else JPC + NI
            nc.gpsimd.tensor_tensor(
                out_sb[:, half : half + NI],
                out_sb[:, half : half + NI],
                dump[:, :],
                mybir.AluOpType.add,
            )


def _aux_consts():
    ob = np.zeros([CHUNK, 64], dtype=np.float16)
    for m in range(KPC):
        ob[5 * m : 5 * m + 5, m] = 2.0
        ob[5 * m : 5 * m + 5, 32 + m] = 1.0
    negI = (-np.eye(128)).astype(np.float16)
    return ob, negI


def make_in_maps(inputs, T):
    f16 = np.float16
    Tm = np.asarray(T, dtype=np.float32).astype(f16)
    ob, negI = _aux_consts()
    in_maps = []
    for c in range(NCORES):
        rolled = np.roll(np.asarray(inputs, dtype=np.float32), -JPC * c, axis=0)
        sl = rolled[:NI].copy()
        if c >= 4:
            # dummy 5th block: pushes |x_i - x_j| to ~1e3+ so e == 0.0
            sl[4 * JPC :] += DUMMY_OFF
        inTc = np.ascontiguousarray(sl.T).astype(f16)
        in_maps.append(
            {
                "inT": inTc,
                "Tm": Tm,
                "onesd": ob,
                "negI": negI,
            }
        )
    return in_maps


def assemble_output(results):
    out = np.zeros([B, K], dtype=np.float32)
    for c in range(NCORES):
        arr = np.asarray(results[c]["out"], dtype=np.float32)  # [128, 64+2*NI]
        colp = arr[:, JPC : JPC + NI] + arr[:, JPC + NI : JPC + 2 * NI]
        g = (JPC * c + np.arange(JPC, NI)) % B
        for cc in range(NCHUNK):
            rows = slice(32 * cc, 32 * cc + KPC)
            ks = slice(KPC * cc, KPC * (cc + 1))
            # row sums of core c cover i-blocks c..c+4
            out[JPC * c : JPC * (c + 1), ks] += arr[rows, 0:JPC].T
            # column partials cover the rows of blocks c+1..c+4
            out[g, ks] += colp[rows, JPC:NI].T
    return out


def kernel(inputs, T):
    from concourse.bass_utils import run_bass_kernel_spmd

    if "nc" not in _NC_CACHE:
        _NC_CACHE["nc"] = build_nc()
    nc = _NC_CACHE["nc"]
    in_maps = make_in_maps(inputs, T)
    res = run_bass_kernel_spmd(nc, in_maps, list(range(NCORES)))
    return assemble_output(res.results)


if __name__ == "__main__":
    sys.path.insert(0, "/root/problem")
    from reference import setup_inputs, reference

    inputs = setup_inputs()
    expected = np.asarray(reference(**inputs))
    actual = kernel(**{k: np.asarray(v) for k, v in inputs.items()})
    err = np.abs(actual - expected)
    rel = np.linalg.norm(actual - expected) / np.linalg.norm(expected)
    print(f"max abs err: {err.max():.3e}")
    print(f"Relative error: {rel:.3e}")


# revision 23
# speedup vs baseline: 1.7419x; 1.1117x over previous
"""
MinibatchDiscrimination kernel for 8x TRN2 NeuronCores (Bass/Tile).

Math:  x = inputs @ T  -> [B, K, D] with B=512, K=100, D=5
       out[i,k] = sum_j exp(-sum_d |x[i,k,d]-x[j,k,d]|)

Strategy -- symmetric half-coverage (e_ij = e_ji):
  The batch is split into 8 blocks of 64 rows. Core c owns output rows
  j in block c, but computes e_ij only for i in blocks c..c+4 (cyclic),
  NI=320 columns instead of 512. Every unordered block pair {a, b} is
  covered exactly once: pairs at cyclic distance d in 1..3 by the lower
  core, d=4 pairs by cores 0..3 only -- cores 4..7 receive +1024-offset
  dummy data for their 5th block so those e values underflow to exactly
  0.0 (their row sums gain nothing, their d=4 column partials are 0).
  Each core emits BOTH row sums (ACT accum_out over the free axis) and
  per-column partial sums over its 64 rows (elementwise accumulation of
  the exp tiles); the host combines: out[rows of block c] = row sums of
  core c + column partials of cores c-1, c-2, c-3 (and c-4 for c>=4)
  at the matching columns. Within-block (d=0) pairs are fully covered by
  the row sums; the d=0 column-partial slice is simply not used.

Per-core device program (SPMD via host-side roll of the batch axis):
  - xT[kd, i] = sum_f T[f, kd] * inputsT[f, i]  (PE, 4 chunks of 125, NI cols)
  - S[k, i] = sum_d x[i,k,d]  (PE 1.0-ones-block matmuls, fp16 S16)
  - per j in 0..63, using |t| = 2*relu(t) - t:
      dist = -S_i (negI matmul) + 2*sum_d relu(x_i - x_j)
             (DVE tensor_scalar (subtract, max 0.0) per chunk + PE
              2.0-ones-block col-tiled matmuls; the per-partition scalar
              is an f32 upcast of the fp16 xT column so the diagonal is
              exactly 0)
      e[:, i] = exp(-dist - S_j)  via ACT exp with scale=-1 and
              per-partition bias AP = -S16[:, j] (f32); since
              sum_d |diff| = 2*sum_d relu - S_i + S_j the S terms cancel
              exactly on the diagonal (self term = exp(0) = 1).
      row sums: ACT accum_out -> out_sb[:, j]
      col partials: out_sb[:, 64:384]  += e  (Pool/GPSIMD, even j)
                    out_sb[:, 384:704] += e  (DVE, odd j)
              two independent same-engine accumulation chains; the host
              adds the two halves.
  - dist row p=32c+m holds k=25c+m (m<25); host transposes/reassembles.

  Cost-model balance per j (NI=320): PE = negI + 4 d-sum matmuls ~670ns,
  ACT = exp+accum ~640ns, DVE = 4 tensor_scalar (4x_2p) + odd-j col
  accum ~690ns, Pool ~365ns avg. The xT/S input stage is ~4.8us of PE.

  Hardware notes baked into the structure (measured on TRN2):
  - Compute instructions carry at most ONE semaphore wait after bacc's
    split pass; persistent manually-rotated tiles (dist/dump/ab) keep
    cross-iteration WAR deps same-engine so waits stay within budget.
"""

import sys
import numpy as np

for _p in ("/opt/trn_rl_repo",):
    if _p not in sys.path:
        sys.path.insert(0, _p)

B = 512
F = 1024
K = 100
D = 5
KD = K * D  # 500
NCORES = 8
JPC = B // NCORES  # 64 output rows per core
NBLK = 5  # i-blocks covered per core
NI = NBLK * JPC  # 320 i columns per core
NCHUNK = 4  # kd chunks of 125
CHUNK = KD // NCHUNK  # 125
KPC = K // NCHUNK  # 25 k's per chunk
DUMMY_OFF = 1024.0  # offset applied to block-4 inputs on cores 4..7

_NC_CACHE = {}


def build_nc(bench_reps=1, ablate=()):
    import contextlib

    import concourse.bass as bass
    import concourse.bacc as bacc
    import concourse.mybir as mybir
    from concourse.tile import TileContext

    nc = bacc.Bacc(None, target_bir_lowering=False, debug=True)

    inT = nc.declare_dram_parameter("inT", [F, NI], mybir.dt.float16, isOutput=False)
    Tm = nc.declare_dram_parameter("Tm", [F, KD], mybir.dt.float16, isOutput=False)
    # [:, 0:32] 2.0-valued d-sum block, [:, 32:64] 1.0-valued d-sum block
    onesd = nc.declare_dram_parameter(
        "onesd", [CHUNK, 64], mybir.dt.float16, isOutput=False
    )
    negI = nc.declare_dram_parameter("negI", [128, 128], mybir.dt.float16, isOutput=False)
    # [:, 0:64] row sums; [:, 64:384] even-j col partials; [:, 384:704] odd-j
    out = nc.declare_dram_parameter(
        "out", [128, JPC + 2 * NI], mybir.dt.float32, isOutput=True
    )

    with TileContext(nc) as tc:
        with tc.tile_pool(name="persist", bufs=1) as pp:
            T_sb = pp.tile([128, 8 * KD], mybir.dt.float16, name="T_sb")
            inT_sb = pp.tile([128, 8 * NI], mybir.dt.float16, name="inT_sb")
            ones_sb = pp.tile([CHUNK, 64], mybir.dt.float16, name="ones_sb")
            negI_sb = pp.tile([128, 128], mybir.dt.float16, name="negI_sb")
            out_sb = pp.tile([128, JPC + 2 * NI], mybir.dt.float32, name="out_sb")
            xT_sb = pp.tile([128, NCHUNK * NI], mybir.dt.float16, name="xT_sb")
            # f32 upcasts of xT columns 0..JPC (tensor_scalar per-partition
            # scalars must be f32). Upcast from the fp16 xT so the diagonal
            # subtract is exactly zero.
            xTj_sb = pp.tile([128, NCHUNK * JPC], mybir.dt.float32, name="xTj_sb")
            S16_sb = pp.tile([128, NI], mybir.dt.float16, name="S16_sb")
            # -S16[:, 0:64] upcast to f32: per-partition exp bias APs
            negSj_sb = pp.tile([128, JPC], mybir.dt.float32, name="negSj_sb")

            # warm the ACT exp table while DMAs run (table load ~1.3us)
            warm_sb = pp.tile([1, 1], mybir.dt.float32, name="warm_sb")
            nc.vector.memset(warm_sb[:, :], 0.0)
            nc.scalar.activation(
                warm_sb[:, :], warm_sb[:, :], mybir.ActivationFunctionType.Exp
            )
            # zero the two column-partial accumulator regions
            nc.gpsimd.memset(out_sb[:, JPC : JPC + 2 * NI], 0.0)

            # --- load inputs ---
            # T/inT in 2-f-tile groups, interleaved, so the PE can start
            # consuming f-tile pairs while later pairs still transfer (each
            # HWDGE issue is ~625ns on a device-global queue, so fewer DMAs
            # beat per-tile ones). The tiny consts go via the Pool SWDGE
            # path, which doesn't contend with HWDGE.
            for g in range(4):
                t0 = 2 * g
                nc.sync.dma_start(
                    out=T_sb[:, t0 * KD : (t0 + 2) * KD].rearrange(
                        "p (g c) -> p g c", g=2
                    ),
                    in_=Tm[t0 * 128 : (t0 + 2) * 128, :].rearrange(
                        "(g p) c -> p g c", g=2
                    ),
                )
                nc.sync.dma_start(
                    out=inT_sb[:, t0 * NI : (t0 + 2) * NI].rearrange(
                        "p (g c) -> p g c", g=2
                    ),
                    in_=inT[t0 * 128 : (t0 + 2) * 128, :].rearrange(
                        "(g p) c -> p g c", g=2
                    ),
                )
            nc.gpsimd.dma_start(out=ones_sb[:, :], in_=onesd[:, :])
            nc.gpsimd.dma_start(out=negI_sb[:, :], in_=negI[:, :])

            with tc.tile_pool(name="xtps", bufs=1, space="PSUM") as xtps:
                # --- xT chunks: xT[kd, i] via PE, f-tile-group outer so the
                # PE consumes DMA groups as they land ---
                xt_ps_l = [
                    xtps.tile([CHUNK, NI], mybir.dt.float32, name=f"xt_ps{c}")
                    for c in range(NCHUNK)
                ]
                S_ps = xtps.tile([128, NI], mybir.dt.float32, name="S_ps", bufs=1)
                for g in range(4):
                    for c in range(NCHUNK):
                        for dt_ in range(2):
                            t = 2 * g + dt_
                            nc.tensor.matmul(
                                xt_ps_l[c][:, :],
                                T_sb[:, t * KD + c * CHUNK : t * KD + (c + 1) * CHUNK],
                                inT_sb[:, t * NI : (t + 1) * NI],
                                start=(t == 0),
                                stop=(t == 7),
                            )
                        if g == 3:
                            # chunk c complete: copy out + S row-sums while
                            # later chunks finish (alternate ACT/DVE to halve
                            # the serial copy tail)
                            if c % 2 == 1:
                                nc.scalar.copy(
                                    xT_sb[0:CHUNK, c * NI : (c + 1) * NI],
                                    xt_ps_l[c][:, :],
                                )
                            else:
                                nc.vector.tensor_copy(
                                    xT_sb[0:CHUNK, c * NI : (c + 1) * NI],
                                    xt_ps_l[c][:, :],
                                )
                            nc.vector.tensor_copy(
                                xTj_sb[0:CHUNK, c * JPC : (c + 1) * JPC],
                                xT_sb[0:CHUNK, c * NI : c * NI + 2 * JPC : 2],
                            )
                            # S[k, i] = sum_d x[i,k,d] at partitions 32c+m
                            nc.tensor.matmul(
                                S_ps[32 * c : 32 * c + 32, :],
                                ones_sb[:, 32:64],
                                xT_sb[0:CHUNK, c * NI : (c + 1) * NI],
                                start=True,
                                stop=True,
                                tile_position=(0, 32 * c),
                            )
                nc.scalar.copy(S16_sb[:, :], S_ps[:, :])
                # f32 upcast of -S16[:, j]: exact negation of the fp16 values
                # so exp(-dist - S_j) is exactly exp(0)=1 on the diagonal.
                nc.vector.tensor_scalar(
                    negSj_sb[:, :],
                    S16_sb[:, 0 : 2 * JPC : 2],
                    -1.0,
                    None,
                    mybir.AluOpType.mult,
                )

            mainps_es = contextlib.ExitStack()
            mainps = mainps_es.enter_context(
                tc.tile_pool(name="mainps", bufs=1, space="PSUM")
            )

            # Persistent, manually double-buffered psum tiles. Persistent
            # (vs pool-rotated) so cross-iteration WAR deps are plain data
            # deps on fixed tiles: same-engine deps then cost no semaphore,
            # which matters because instructions carry at most ONE wait.
            dist_bufs = [
                mainps.tile([128, NI], mybir.dt.float32, name=f"dist{i}")
                for i in range(4)
            ]
            dump_bufs = [
                pp.tile([128, NI], mybir.dt.float32, name=f"dump{i}") for i in range(2)
            ]
            # Persistent relu tiles, manually rotated (same reason).
            NAB = 8
            ab_bufs = [
                pp.tile([CHUNK, NI], mybir.dt.float16, name=f"ab{i}") for i in range(NAB)
            ]

            rep_ctx = (
                tc.For_i(0, bench_reps, 1) if bench_reps > 1 else contextlib.nullcontext()
            )
            with rep_ctx:
                main_loop(nc, mybir, xT_sb, xTj_sb, S16_sb, negSj_sb, negI_sb,
                          ones_sb, out_sb, dist_bufs, dump_bufs, ab_bufs, ablate)

            mainps_es.close()
            # split output DMA: the even-j col-partial half is complete after
            # j=62, so its transfer overlaps the last iteration
            nc.sync.dma_start(
                out=out[:, JPC + NI :], in_=out_sb[:, JPC + NI :]
            )
            nc.sync.dma_start(
                out=out[:, : JPC + NI], in_=out_sb[:, : JPC + NI]
            )

    nc.finalize()
    return nc


def main_loop(nc, mybir, xT_sb, xTj_sb, S16_sb, negSj_sb, negI_sb, ones_sb,
              out_sb, dist_bufs, dump_bufs, ab_bufs, ablate=()):
    NAB = len(ab_bufs)
    for j in range(JPC):
        # triangle trim: i-window [2j, NI) over the interleaved layout (own
        # col j at 2j, pair-block col at 2j+1). Skipped pairs are recovered
        # from the column partials of this core (own-block lower triangle)
        # and of the pair core (complementary block-4 triangle); the
        # diagonal is excluded from the col accum.
        lo = 2 * j
        dist = dist_bufs[j % 4]
        for c in range(NCHUNK):
            ab = ab_bufs[(j * NCHUNK + c) % NAB]
            if "ts" not in ablate:
                # relu(x_i - x_j) = (x_i - x_j) max 0; const scalar2
                # keeps the second DVE read port free for 4x_2p mode.
                nc.vector.tensor_scalar(
                    ab[:, lo:NI],
                    xT_sb[0:CHUNK, c * NI + lo : (c + 1) * NI],
                    xTj_sb[0:CHUNK, c * JPC + j : c * JPC + j + 1],
                    0.0,
                    mybir.AluOpType.subtract,
                    mybir.AluOpType.max,
                )
            # dist[32c+m, :] += 2 * sum_d ab[5m+d, :]
            if "mm" not in ablate:
                nc.tensor.matmul(
                    dist[32 * c : 32 * c + 32, lo:NI],
                    ones_sb[:, 0:32],
                    ab[:, lo:NI],
                    start=True,
                    stop=False,
                    tile_position=(0, 32 * c),
                    skip_group_check=True,
                )
        # dist += -S[k, i] (all 128 rows), closing the accumulation; last
        # so the first iterations don't serialize behind the S16 stage
        if "mm" not in ablate:
            nc.tensor.matmul(
                dist[:, lo:NI],
                negI_sb[:, :],
                S16_sb[:, lo:NI],
                start=False,
                stop=True,
                skip_group_check=True,
            )
        dump = dump_bufs[j % 2]
        # e = exp(-dist - S_j); row sums over i in [2j, NI) via accum_out
        if "exp" not in ablate:
            nc.scalar.activation(
                dump[:, lo:NI],
                dist[:, lo:NI],
                mybir.ActivationFunctionType.Exp,
                bias=negSj_sb[:, j : j + 1],
                scale=-1.0,
                accum_out=(None if "noaccum" in ablate else out_sb[:, j : j + 1]),
            )
        # column partials over i in [2j+1, NI) -- diagonal excluded: two
        # independent Pool accumulation chains (Pool tensor_tensor is
        # ~0.83ns/col in the cost model and the engine is otherwise idle).
        # Even j's accumulate into the [JPC+NI:] half so that region is
        # finished one iteration early and its output DMA can overlap the
        # last iteration.
        if "colacc" not in ablate:
            half = JPC + NI if j % 2 == 0 else JPC
            nc.gpsimd.tensor_tensor(
                out_sb[:, half + lo + 1 : half + NI],
                out_sb[:, half + lo + 1 : half + NI],
                dump[:, lo + 1 : NI],
                mybir.AluOpType.add,
            )


def _aux_consts():
    ob = np.zeros([CHUNK, 64], dtype=np.float16)
    for m in range(KPC):
        ob[5 * m : 5 * m + 5, m] = 2.0
        ob[5 * m : 5 * m + 5, 32 + m] = 1.0
    negI = (-np.eye(128)).astype(np.float16)
    return ob, negI


def _col_layout(c):
    """Local column -> global batch row for core c, plus dummy mask.

    Interleaved layout: col 2t = own-block row t, col 2t+1 = pair-block
    (c+4) row; cols 128.. = blocks c+1..c+3. The pair block {c, c+4} is
    split between its two owners by the triangle window [2j, NI): core c
    covers b4 rows t >= j, core c+4 (whose b4 columns are shifted by one:
    col 2t+1 = row t+1 of block c-4) covers the complement. Its t=63
    column has no row left to hold -- it is a dummy (offset) column.
    """
    g = np.zeros(NI, dtype=np.int64)
    dummy = np.zeros(NI, dtype=bool)
    t = np.arange(JPC)
    g[2 * t] = JPC * c + t
    if c < 4:
        g[2 * t + 1] = JPC * (c + 4) + t
    else:
        g[2 * t + 1] = JPC * (c + 4) + t + 1
        dummy[2 * 63 + 1] = True
    g[2 * JPC :] = JPC * (c + 1) + np.arange(2 * JPC, NI) - 2 * JPC
    return g % B, dummy


def make_in_maps(inputs, T):
    f16 = np.float16
    Tm = np.asarray(T, dtype=np.float32).astype(f16)
    ob, negI = _aux_consts()
    in_maps = []
    inp = np.asarray(inputs, dtype=np.float32)
    for c in range(NCORES):
        g, dummy = _col_layout(c)
        sl = inp[g].copy()
        # dummy column: pushes |x_i - x_j| to ~1e3+ so e == 0.0 exactly
        sl[dummy] += DUMMY_OFF
        inTc = np.ascontiguousarray(sl.T).astype(f16)
        in_maps.append(
            {
                "inT": inTc,
                "Tm": Tm,
                "onesd": ob,
                "negI": negI,
            }
        )
    return in_maps


def assemble_output(results):
    out = np.zeros([B, K], dtype=np.float32)
    for c in range(NCORES):
        arr = np.asarray(results[c]["out"], dtype=np.float32)  # [128, 64+2*NI]
        colp = arr[:, JPC : JPC + NI] + arr[:, JPC + NI : JPC + 2 * NI]
        g, _ = _col_layout(c)
        for cc in range(NCHUNK):
            rows = slice(32 * cc, 32 * cc + KPC)
            ks = slice(KPC * cc, KPC * (cc + 1))
            # row sums of core c: i in [2j, NI)
            out[JPC * c : JPC * (c + 1), ks] += arr[rows, 0:JPC].T
            # column partials: the complementary triangles (dummy col is 0)
            np.add.at(out[:, ks], g, colp[rows, :].T)
    return out


def kernel(inputs, T):
    from concourse.bass_utils import run_bass_kernel_spmd

    if "nc" not in _NC_CACHE:
        _NC_CACHE["nc"] = build_nc()
    nc = _NC_CACHE["nc"]
    in_maps = make_in_maps(inputs, T)
    res = run_bass_kernel_spmd(nc, in_maps, list(range(NCORES)))
    return assemble_output(res.results)


if __name__ == "__main__":
    sys.path.insert(0, "/root/problem")
    from reference import setup_inputs, reference

    inputs = setup_inputs()
    expected = np.asarray(reference(**inputs))
    actual = kernel(**{k: np.asarray(v) for k, v in inputs.items()})
    err = np.abs(actual - expected)
    rel = np.linalg.norm(actual - expected) / np.linalg.norm(expected)
    print(f"max abs err: {err.max():.3e}")
    print(f"Relative error: {rel:.3e}")


# revision 38
# speedup vs baseline: 1.7638x; 1.0126x over previous
"""
MinibatchDiscrimination kernel for 8x TRN2 NeuronCores (Bass/Tile).

Math:  x = inputs @ T  -> [B, K, D] with B=512, K=100, D=5
       out[i,k] = sum_j exp(-sum_d |x[i,k,d]-x[j,k,d]|)

Strategy -- symmetric near-exact-half coverage (e_ij = e_ji):
  The batch is split into 8 blocks of 64 rows. Core c owns output rows
  j in block c and holds NI=320 i-columns in an interleaved order (see
  _col_layout): cols 2t / 2t+1 = own-block row t / pair-block (c+4) row,
  cols 128.. = blocks c+1..c+3. Each j processes only the column window
  [2j, NI), which covers every unordered row pair exactly once across
  the fleet (~99.8% efficiency):
  - blocks at cyclic distance 1..3: fully by core c (no other core has
    those pairs);
  - own block: upper triangle via the window, lower triangle recovered
    from this core's column partials (column 2t accumulates exactly the
    rows j < t; the diagonal is excluded from the column accumulation);
  - pair block {c, c+4}: split between its two owners by the same
    window. Core c+4's pair-block columns are shifted by one row
    (col 2t+1 = row t+1 of block c-4), which makes the two windows
    exactly complementary; its t=63 column has no row left and carries
    +1024-offset dummy data so its e values underflow to exactly 0.0.
  Each core emits row sums (ACT accum_out over the free window) and
  column partials (elementwise accumulation of the exp tiles; two
  chains so the even half can be DMA'd out one iteration early). The
  host maps local columns to global rows via the same _col_layout and
  scatter-adds: out = row sums + column partials.

Per-core device program (SPMD: identical program, per-core input maps):
  - DMA: T/inT in 2-f-tile groups, interleaved, so the PE consumes
    f-tile pairs as they land (transfers serialize on the DMA complex at
    ~360GB/s; 1.64MB is ~4.7us and paces the fill); consts go via the
    Pool SWDGE path which doesn't contend with the HWDGE queue.
  - xT[kd, i] = sum_f T[f, kd] * inputsT[f, i]  (PE, 4 chunks of 125,
    f-tile-group outer; per-chunk copies to fp16 alternate ACT/DVE)
  - S[k, i] = sum_d x[i,k,d]  (PE 1.0-ones-block matmuls, fp16 S16)
  - per j in 0..63, window W = [2j, NI), using |t| = 2*relu(t) - t:
      ab_c = relu(x_i - x_j)   (DVE tensor_scalar (subtract, max 0.0),
              4x_2p mode; the per-partition scalar is an f32 upcast of
              the fp16 xT column so the diagonal is exactly 0)
      dist = 2*sum_d ab (PE 2.0-ones-block col-tiled matmuls, each block
              start=True) then dist += -S_i (negI matmul, start=False,
              stop=True, emitted LAST so early iterations don't
              serialize behind the S16 stage)
      e = exp(-dist - S_j)  via ACT exp, scale=-1, per-partition bias
              AP = -S16[:, 2j] (f32): the S terms cancel exactly on the
              diagonal (self term = exp(0) = 1).
      row sums: ACT accum_out -> out_sb[:, j]
      col partials (diagonal col excluded): out_sb[:, 64+NI:] += e for
              even j, out_sb[:, 64:64+NI] += e for odd j -- two Pool
              (GPSIMD) accumulation chains. GPSIMD cannot access PSUM,
              which is why e dumps to SBUF f32.
  - dist row p=32c+m holds k=25c+m (m<25); host reassembles.

  Cost-model balance (sim 50.4us total): the 64-exp ACT stream is the
  loop pacer and runs back-to-back (~37.5us: W*0.833ns + 372ns fixed
  init/accum overhead per exp); PE is ~2% behind it (negI + 4 d-sum
  matmuls at W*0.4167ns each + 4.8us xT/S input stage), DVE ~34us,
  Pool ~15us. Fill ~9us is DMA-transfer-paced; tail ~3us is the last
  exp -> col accum -> DMA -> completion-semaphore chain.

  Notes from this optimization pass (cost model + real-compiler):
  - abs_max is NOT a legal tensor_scalar ALU op on HW (walrus ISA check
    rejects it; the cost model accepts it) -- hence the 2*relu - S form.
  - GPSIMD cannot access PSUM (BIR verifier).
  - A matmul costs free_size x 0.4167ns (fp16) regardless of partition
    counts; DVE 4x_2p needs all-SBUF 2-byte packed operands; ACT
    activation charges free_size x 0.833ns + memory-access init, and
    accum_out adds a 187ns accumulator-read op.
  - Compute instructions carry at most ONE semaphore wait after bacc's
    split pass; persistent manually-rotated tiles (dist/dump/ab) keep
    cross-iteration WAR deps same-engine so waits stay within budget.
"""

import sys
import numpy as np

for _p in ("/opt/trn_rl_repo",):
    if _p not in sys.path:
        sys.path.insert(0, _p)

B = 512
F = 1024
K = 100
D = 5
KD = K * D  # 500
NCORES = 8
JPC = B // NCORES  # 64 output rows per core
NBLK = 5  # i-blocks covered per core
NI = NBLK * JPC  # 320 i columns per core
NCHUNK = 4  # kd chunks of 125
CHUNK = KD // NCHUNK  # 125
KPC = K // NCHUNK  # 25 k's per chunk
DUMMY_OFF = 1024.0  # offset applied to block-4 inputs on cores 4..7

_NC_CACHE = {}


def build_nc(bench_reps=1, ablate=()):
    import contextlib

    import concourse.bass as bass
    import concourse.bacc as bacc
    import concourse.mybir as mybir
    from concourse.tile import TileContext

    nc = bacc.Bacc(None, target_bir_lowering=False, debug=True)

    inT = nc.declare_dram_parameter("inT", [F, NI], mybir.dt.float16, isOutput=False)
    Tm = nc.declare_dram_parameter("Tm", [F, KD], mybir.dt.float16, isOutput=False)
    # [:, 0:32] 2.0-valued d-sum block, [:, 32:64] 1.0-valued d-sum block
    onesd = nc.declare_dram_parameter(
        "onesd", [CHUNK, 64], mybir.dt.float16, isOutput=False
    )
    negI = nc.declare_dram_parameter("negI", [128, 128], mybir.dt.float16, isOutput=False)
    # [:, 0:64] row sums; [:, 64:384] odd-j col partials; [:, 384:704] even-j
    out = nc.declare_dram_parameter(
        "out", [128, JPC + 2 * NI], mybir.dt.float32, isOutput=True
    )

    with TileContext(nc) as tc:
        with tc.tile_pool(name="persist", bufs=1) as pp:
            T_sb = pp.tile([128, 8 * KD], mybir.dt.float16, name="T_sb")
            inT_sb = pp.tile([128, 8 * NI], mybir.dt.float16, name="inT_sb")
            ones_sb = pp.tile([CHUNK, 64], mybir.dt.float16, name="ones_sb")
            negI_sb = pp.tile([128, 128], mybir.dt.float16, name="negI_sb")
            out_sb = pp.tile([128, JPC + 2 * NI], mybir.dt.float32, name="out_sb")
            xT_sb = pp.tile([128, NCHUNK * NI], mybir.dt.float16, name="xT_sb")
            # f32 upcasts of xT columns 0..JPC (tensor_scalar per-partition
            # scalars must be f32). Upcast from the fp16 xT so the diagonal
            # subtract is exactly zero.
            xTj_sb = pp.tile([128, NCHUNK * JPC], mybir.dt.float32, name="xTj_sb")
            S16_sb = pp.tile([128, NI], mybir.dt.float16, name="S16_sb")
            # -S16[:, 0:64] upcast to f32: per-partition exp bias APs
            negSj_sb = pp.tile([128, JPC], mybir.dt.float32, name="negSj_sb")

            # warm the ACT exp table while DMAs run (table load ~1.3us)
            warm_sb = pp.tile([1, 1], mybir.dt.float32, name="warm_sb")
            nc.vector.memset(warm_sb[:, :], 0.0)
            nc.scalar.activation(
                warm_sb[:, :], warm_sb[:, :], mybir.ActivationFunctionType.Exp
            )
            # zero the two column-partial accumulator regions
            nc.gpsimd.memset(out_sb[:, JPC : JPC + 2 * NI], 0.0)

            # --- load inputs ---
            # T/inT in 2-f-tile groups, interleaved, so the PE can start
            # consuming f-tile pairs while later pairs still transfer (each
            # HWDGE issue is ~625ns on a device-global queue, so fewer DMAs
            # beat per-tile ones). The tiny consts go via the Pool SWDGE
            # path, which doesn't contend with HWDGE.
            for g in range(4):
                t0 = 2 * g
                nc.sync.dma_start(
                    out=T_sb[:, t0 * KD : (t0 + 2) * KD].rearrange(
                        "p (g c) -> p g c", g=2
                    ),
                    in_=Tm[t0 * 128 : (t0 + 2) * 128, :].rearrange(
                        "(g p) c -> p g c", g=2
                    ),
                )
                nc.sync.dma_start(
                    out=inT_sb[:, t0 * NI : (t0 + 2) * NI].rearrange(
                        "p (g c) -> p g c", g=2
                    ),
                    in_=inT[t0 * 128 : (t0 + 2) * 128, :].rearrange(
                        "(g p) c -> p g c", g=2
                    ),
                )
            # consts issue after the 8 data groups: their transfers then
            # queue behind the T/inT chain instead of interleaving into it
            # (transfers serialize on one global DMA complex), and they are
            # not needed until the S matmuls / first negI (~8us)
            nc.sync.dma_start(out=ones_sb[:, :], in_=onesd[:, :])
            nc.sync.dma_start(out=negI_sb[:, :], in_=negI[:, :])

            mainps_es = contextlib.ExitStack()
            mainps = mainps_es.enter_context(
                tc.tile_pool(name="mainps", bufs=1, space="PSUM")
            )

            # Persistent, manually double-buffered psum tiles. Persistent
            # (vs pool-rotated) so cross-iteration WAR deps are plain data
            # deps on fixed tiles: same-engine deps then cost no semaphore,
            # which matters because instructions carry at most ONE wait.
            # dist0/1 are allocated BEFORE the xtps stage pool so their banks
            # are disjoint from xt_ps/S_ps: otherwise dist0's first matmul
            # carries a WAR wait on the S16 copy still reading S_ps. dist2/3
            # reuse freed stage banks (their first use is late enough); only
            # 8 psum banks exist so all four cannot pre-allocate.
            dist_bufs = [
                mainps.tile([128, NI], mybir.dt.float32, name=f"dist{i}")
                for i in range(2)
            ]
            dump_bufs = [
                pp.tile([128, NI], mybir.dt.float32, name=f"dump{i}") for i in range(2)
            ]
            # Persistent relu tiles, manually rotated (same reason).
            NAB = 8
            ab_bufs = [
                pp.tile([CHUNK, NI], mybir.dt.float16, name=f"ab{i}") for i in range(NAB)
            ]

            with tc.tile_pool(name="xtps", bufs=1, space="PSUM") as xtps:
                # --- xT chunks: xT[kd, i] via PE, f-tile-group outer so the
                # PE consumes DMA groups as they land ---
                xt_ps_l = [
                    xtps.tile([CHUNK, NI], mybir.dt.float32, name=f"xt_ps{c}")
                    for c in range(NCHUNK)
                ]
                S_ps = xtps.tile([128, NI], mybir.dt.float32, name="S_ps", bufs=1)
                for g in range(4):
                    for c in range(NCHUNK):
                        for dt_ in range(2):
                            t = 2 * g + dt_
                            nc.tensor.matmul(
                                xt_ps_l[c][:, :],
                                T_sb[:, t * KD + c * CHUNK : t * KD + (c + 1) * CHUNK],
                                inT_sb[:, t * NI : (t + 1) * NI],
                                start=(t == 0),
                                stop=(t == 7),
                            )
                        if g == 3:
                            # chunk c complete: copy out + S row-sums while
                            # later chunks finish (alternate ACT/DVE to halve
                            # the serial copy tail)
                            if c % 2 == 1:
                                nc.scalar.copy(
                                    xT_sb[0:CHUNK, c * NI : (c + 1) * NI],
                                    xt_ps_l[c][:, :],
                                )
                            else:
                                nc.vector.tensor_copy(
                                    xT_sb[0:CHUNK, c * NI : (c + 1) * NI],
                                    xt_ps_l[c][:, :],
                                )
                            nc.gpsimd.tensor_copy(
                                xTj_sb[0:CHUNK, c * JPC : (c + 1) * JPC],
                                xT_sb[0:CHUNK, c * NI : c * NI + 2 * JPC : 2],
                            )
                            # S[k, i] = sum_d x[i,k,d] at partitions 32c+m
                            nc.tensor.matmul(
                                S_ps[32 * c : 32 * c + 32, :],
                                ones_sb[:, 32:64],
                                xT_sb[0:CHUNK, c * NI : (c + 1) * NI],
                                start=True,
                                stop=True,
                                tile_position=(0, 32 * c),
                            )
                nc.scalar.copy(S16_sb[:, :], S_ps[:, :])

            mainps2 = mainps_es.enter_context(
                tc.tile_pool(name="mainps2", bufs=1, space="PSUM")
            )
            dist_bufs += [
                mainps2.tile([128, NI], mybir.dt.float32, name=f"dist{i}")
                for i in range(2, 4)
            ]

            rep_ctx = (
                tc.For_i(0, bench_reps, 1) if bench_reps > 1 else contextlib.nullcontext()
            )
            with rep_ctx:
                main_loop(nc, mybir, xT_sb, xTj_sb, S16_sb, negSj_sb, negI_sb,
                          ones_sb, out_sb, dist_bufs, dump_bufs, ab_bufs, ablate)

            mainps_es.close()
            # split output DMA: the even-j col-partial half is complete after
            # j=62, so its transfer overlaps the last iteration
            nc.sync.dma_start(
                out=out[:, JPC + NI :], in_=out_sb[:, JPC + NI :]
            )
            nc.sync.dma_start(
                out=out[:, : JPC + NI], in_=out_sb[:, : JPC + NI]
            )

    nc.finalize()
    return nc


def main_loop(nc, mybir, xT_sb, xTj_sb, S16_sb, negSj_sb, negI_sb, ones_sb,
              out_sb, dist_bufs, dump_bufs, ab_bufs, ablate=()):
    NAB = len(ab_bufs)
    for j in range(JPC):
        # triangle trim: i-window [2j, NI) over the interleaved layout (own
        # col j at 2j, pair-block col at 2j+1). Skipped pairs are recovered
        # from the column partials of this core (own-block lower triangle)
        # and of the pair core (complementary block-4 triangle); the
        # diagonal is excluded from the col accum.
        lo = 2 * j
        dist = dist_bufs[j % 4]
        for c in range(NCHUNK):
            ab = ab_bufs[(j * NCHUNK + c) % NAB]
            if "ts" not in ablate:
                # relu(x_i - x_j) = (x_i - x_j) max 0; const scalar2
                # keeps the second DVE read port free for 4x_2p mode.
                nc.vector.tensor_scalar(
                    ab[:, lo:NI],
                    xT_sb[0:CHUNK, c * NI + lo : (c + 1) * NI],
                    xTj_sb[0:CHUNK, c * JPC + j : c * JPC + j + 1],
                    0.0,
                    mybir.AluOpType.subtract,
                    mybir.AluOpType.max,
                )
            # dist[32c+m, :] += 2 * sum_d ab[5m+d, :]
            if "mm" not in ablate:
                nc.tensor.matmul(
                    dist[32 * c : 32 * c + 32, lo:NI],
                    ones_sb[:, 0:32],
                    ab[:, lo:NI],
                    start=True,
                    stop=False,
                    tile_position=(0, 32 * c),
                    skip_group_check=True,
                )
        if j == 0:
            # f32 upcast of -S16[:, j]: exact negation of the fp16 values so
            # exp(-dist - S_j) is exactly exp(0)=1 on the diagonal. Emitted
            # after j0's tensor_scalars so they don't queue behind the S16
            # wait on the in-order DVE queue.
            nc.gpsimd.tensor_scalar(
                negSj_sb[:, :],
                S16_sb[:, 0 : 2 * JPC : 2],
                -1.0,
                None,
                mybir.AluOpType.mult,
            )
        # dist += -S[k, i] (all 128 rows), closing the accumulation; last
        # so the first iterations don't serialize behind the S16 stage
        if "mm" not in ablate:
            nc.tensor.matmul(
                dist[:, lo:NI],
                negI_sb[:, :],
                S16_sb[:, lo:NI],
                start=False,
                stop=True,
                skip_group_check=True,
            )
        dump = dump_bufs[j % 2]
        # e = exp(-dist - S_j); row sums over i in [2j, NI) via accum_out
        if "exp" not in ablate:
            nc.scalar.activation(
                dump[:, lo:NI],
                dist[:, lo:NI],
                mybir.ActivationFunctionType.Exp,
                bias=negSj_sb[:, j : j + 1],
                scale=-1.0,
                accum_out=(None if "noaccum" in ablate else out_sb[:, j : j + 1]),
            )
        # column partials over i in [2j+1, NI) -- diagonal excluded: two
        # independent Pool accumulation chains (Pool tensor_tensor is
        # ~0.83ns/col in the cost model and the engine is otherwise idle).
        # Even j's accumulate into the [JPC+NI:] half so that region is
        # finished one iteration early and its output DMA can overlap the
        # last iteration.
        if "colacc" not in ablate:
            half = JPC + NI if j % 2 == 0 else JPC
            nc.gpsimd.tensor_tensor(
                out_sb[:, half + lo + 1 : half + NI],
                out_sb[:, half + lo + 1 : half + NI],
                dump[:, lo + 1 : NI],
                mybir.AluOpType.add,
            )


def _aux_consts():
    ob = np.zeros([CHUNK, 64], dtype=np.float16)
    for m in range(KPC):
        ob[5 * m : 5 * m + 5, m] = 2.0
        ob[5 * m : 5 * m + 5, 32 + m] = 1.0
    negI = (-np.eye(128)).astype(np.float16)
    return ob, negI


def _col_layout(c):
    """Local column -> global batch row for core c, plus dummy mask.

    Interleaved layout: col 2t = own-block row t, col 2t+1 = pair-block
    (c+4) row; cols 128.. = blocks c+1..c+3. The pair block {c, c+4} is
    split between its two owners by the triangle window [2j, NI): core c
    covers b4 rows t >= j, core c+4 (whose b4 columns are shifted by one:
    col 2t+1 = row t+1 of block c-4) covers the complement. Its t=63
    column has no row left to hold -- it is a dummy (offset) column.
    """
    g = np.zeros(NI, dtype=np.int64)
    dummy = np.zeros(NI, dtype=bool)
    t = np.arange(JPC)
    g[2 * t] = JPC * c + t
    if c < 4:
        g[2 * t + 1] = JPC * (c + 4) + t
    else:
        g[2 * t + 1] = JPC * (c + 4) + t + 1
        dummy[2 * 63 + 1] = True
    g[2 * JPC :] = JPC * (c + 1) + np.arange(2 * JPC, NI) - 2 * JPC
    return g % B, dummy


def make_in_maps(inputs, T):
    f16 = np.float16
    Tm = np.asarray(T, dtype=np.float32).astype(f16)
    ob, negI = _aux_consts()
    in_maps = []
    inp = np.asarray(inputs, dtype=np.float32)
    for c in range(NCORES):
        g, dummy = _col_layout(c)
        sl = inp[g].copy()
        # dummy column: pushes |x_i - x_j| to ~1e3+ so e == 0.0 exactly
        sl[dummy] += DUMMY_OFF
        inTc = np.ascontiguousarray(sl.T).astype(f16)
        in_maps.append(
            {
                "inT": inTc,
                "Tm": Tm,
                "onesd": ob,
                "negI": negI,
            }
        )
    return in_maps


def assemble_output(results):
    out = np.zeros([B, K], dtype=np.float32)
    for c in range(NCORES):
        arr = np.asarray(results[c]["out"], dtype=np.float32)  # [128, 64+2*NI]
        colp = arr[:, JPC : JPC + NI] + arr[:, JPC + NI : JPC + 2 * NI]
        g, _ = _col_layout(c)
        for cc in range(NCHUNK):
            rows = slice(32 * cc, 32 * cc + KPC)
            ks = slice(KPC * cc, KPC * (cc + 1))
            # row sums of core c: i in [2j, NI)
            out[JPC * c : JPC * (c + 1), ks] += arr[rows, 0:JPC].T
            # column partials: the complementary triangles (dummy col is 0)
            np.add.at(out[:, ks], g, colp[rows, :].T)
    return out


def kernel(inputs, T):
    from concourse.bass_utils import run_bass_kernel_spmd

    if "nc" not in _NC_CACHE:
        _NC_CACHE["nc"] = build_nc()
    nc = _NC_CACHE["nc"]
    in_maps = make_in_maps(inputs, T)
    res = run_bass_kernel_spmd(nc, in_maps, list(range(NCORES)))
    return assemble_output(res.results)


if __name__ == "__main__":
    sys.path.insert(0, "/root/problem")
    from reference import setup_inputs, reference

    inputs = setup_inputs()
    expected = np.asarray(reference(**inputs))
    actual = kernel(**{k: np.asarray(v) for k, v in inputs.items()})
    err = np.abs(actual - expected)
    rel = np.linalg.norm(actual - expected) / np.linalg.norm(expected)
    print(f"max abs err: {err.max():.3e}")
    print(f"Relative error: {rel:.3e}")
